# revision 1
# baseline (speedup 1.0000x reference)
"""Trainium2 Bass kernel for nn_AttentionEvaluatorModel (8-core SPMD, data-parallel over batch).

Math (reference):
    qm  = (query @ Wq1.T + bq1) @ Wq2.T + bq2                     (B, Q, E)
    fm  = (features @ Wf1.T + bf1) @ Wf2.T + bf2                  (B, F, E)
    wts = sigmoid(qm @ fm.T) * (ftw * mask)                       (B, Q, F)
    out = cls(wts @ values)                                       (B, Q, L)

Restructure: both mappers are affine (no nonlinearity) and fm only appears
inside qm @ fm.T, so the whole query/feature mapper chain folds (float64, on
host) into one effective per-token logit weight:
    qm @ fm.T = qmw @ features.T + s 1^T
    qmw = query @ (Wq1.T Wq2.T Wf2 Wf1) + (bq1 Wq2.T + bq2) Wf2 Wf1   (B*Q, FS)
    s   = query @ (Wq1.T Wq2.T sv) + (bq1 Wq2.T + bq2) sv, sv = Wf2 bf1 + bf2
This removes the (B,F,FS)x(FS,H)x(H,E) feature-mapper matmuls and ~10 MB of
per-core weight DMA. The kernel is then memory-bound on streaming
features+values, which are host-cast to bf16 (halving HBM traffic) and
host-pre-tiled/transposed so every DMA is one instruction with >=1KB
contiguous runs and zero on-chip transposes.

Each of the 8 cores handles B/8 = 2 batches end-to-end; no inter-core comms.

Per-core on-chip dataflow (TOK = 2 batches x 64 queries = 128):
    stream 2x4 groups of [512 fs, 1024 F] features + [1024 F, 512 E] values:
      per 128-F chunk: logits[F,q] = ftT.T @ qmwT   (4 bf16 mm, f32 PSUM)
                       sigmoid (ACT) -> * gate[F]   (DVE, bf16 out)
                       pooledT[e,q] += vl.T @ wts   (4 bf16 mm, f32 PSUM)
    cls in fully transposed orientation (no PE transposes):
      hT[ch,q] = Wc1-blocks.T @ pooledT (accumulated over two pooled halves,
      half overlapping the stream), relu (ACT), out[q,L] = hT.T @ Wc2T.
    The last feature/value group is issued as smaller DMA pieces so the end
    drain after the final bytes is short.

PSUM note: interleaved per-slice accumulation groups clobber each other via
matmul start=True (it resets more than the instruction's own output columns),
so long-lived accumulators (pooledT, hT) are zeroed once with memset and all
their matmuls run start=False + skip_group_check.
"""

import numpy as np
import ml_dtypes
from contextlib import ExitStack

from concourse import bass, bacc, tile, mybir
from concourse.bass_utils import run_bass_kernel_spmd

P = 128
N_CORES = 8
B, Q, F, E = 16, 64, 4096, 512
QS, FS, H, CH, L = 512, 512, 1024, 1024, 128
BPC = B // N_CORES          # batches per core (2)
TOK = BPC * Q               # tokens per core (128)
FCH = F // P                # feature chunks per batch (32)
NG = 4                      # feature groups per batch
FPG = F // NG               # F-rows per group
GCH = FCH // NG             # chunks per group (8)
KQ = QS // P                # 4 contraction blocks over QS
KF = FS // P                # 4 contraction blocks over FS

f32 = mybir.dt.float32
bf16 = mybir.dt.bfloat16
bfnp = ml_dtypes.bfloat16

_GRAPH_CACHE = {}


def _build(nzb: bool):
    """Build the SPMD single-core graph. nzb: whether bias vectors are nonzero."""
    nc = bacc.Bacc("TRN2", target_bir_lowering=False, debug=False,
                   num_devices=N_CORES)

    # host-pre-tiled inputs (see run() for layouts)
    ft_d = nc.dram_tensor("ftT", (BPC * NG, P, KF, FPG), bf16, kind="ExternalInput").ap()
    vl_d = nc.dram_tensor("vals", (BPC * NG, P, GCH, E), bf16, kind="ExternalInput").ap()
    qmwt_d = nc.dram_tensor("qmwT", (P, KF * TOK), bf16, kind="ExternalInput").ap()
    gt_d = nc.dram_tensor("gateT", (P, BPC * FCH), f32, kind="ExternalInput").ap()
    w1_d = nc.dram_tensor("Wc1T", (P, (E // P) * CH), bf16, kind="ExternalInput").ap()
    w2_d = nc.dram_tensor("Wc2T", (P, (CH // P) * L), bf16, kind="ExternalInput").ap()
    if nzb:
        s_d = nc.dram_tensor("srow", (1, TOK), bf16, kind="ExternalInput").ap()
        bc1_d = nc.dram_tensor("bc1", (1, CH), bf16, kind="ExternalInput").ap()
        bc2_d = nc.dram_tensor("bc2", (1, L), bf16, kind="ExternalInput").ap()
    out_d = nc.dram_tensor("out", (TOK, L), f32, kind="ExternalOutput").ap()

    with tile.TileContext(nc) as tc, ExitStack() as ctx:
        const = ctx.enter_context(tc.tile_pool(name="const", bufs=1))
        wset = ctx.enter_context(tc.tile_pool(name="wset", bufs=1))
        ftp = ctx.enter_context(tc.tile_pool(name="ftp", bufs=4))
        valp = ctx.enter_context(tc.tile_pool(name="valp", bufs=4))
        sigp = ctx.enter_context(tc.tile_pool(name="sigp", bufs=4))
        wtsp = ctx.enter_context(tc.tile_pool(name="wtsp", bufs=4))
        clsp = ctx.enter_context(tc.tile_pool(name="clsp", bufs=2))
        aux_ps = ctx.enter_context(tc.tile_pool(name="aux_ps", bufs=1, space="PSUM"))
        lg_ps = ctx.enter_context(tc.tile_pool(name="lg_ps", bufs=4, space="PSUM"))
        pool_ps = ctx.enter_context(tc.tile_pool(name="pool_ps", bufs=3, space="PSUM"))

        # ---- setup DMAs (ordered by first use: qmwT first, then the first
        # feature groups, gate; cls weights much later) ------------------------
        qmwT = wset.tile([P, KF * TOK], bf16)
        nc.gpsimd.dma_start(qmwT[:], qmwt_d[:])
        # stream pools + group DMA issue (the first groups go out right after
        # queryT/C so the feature stream owns the DMA engines from the start)
        GSKEW = 3
        ngroups = BPC * NG
        st = {}

        HG = GCH // 2

        def issue_group(gi):
            if gi >= ngroups:
                return
            if gi < ngroups - 1:
                ft = ftp.tile([P, KF, FPG], bf16, tag="ft")
                nc.sync.dma_start(ft[:], ft_d[gi])
                vl = valp.tile([P, GCH, E], bf16, tag="vl")
                nc.sync.dma_start(vl[:], vl_d[gi])
                st[gi] = ((ft, ft), (vl,), False)
            else:
                # split the last group so the final chunks' tiles (and their
                # DMA semaphores) land earlier, shortening the end drain
                fta = ftp.tile([P, KF, FPG // 2], bf16, tag="ft", name="fta")
                nc.sync.dma_start(fta[:], ft_d[gi, :, :, :FPG // 2])
                pieces = [2, 2, 2, 1, 1]
                vls = []
                vmap = []
                cg0 = 0
                for pi, n in enumerate(pieces):
                    if pi == 1:
                        ftb = ftp.tile([P, KF, FPG // 2], bf16, tag="ft",
                                       name="ftb")
                        nc.sync.dma_start(ftb[:], ft_d[gi, :, :, FPG // 2:])
                    v = valp.tile([P, n, E], bf16, tag="vl", name=f"vl{pi}")
                    nc.sync.dma_start(v[:], vl_d[gi, :, cg0:cg0 + n, :])
                    for i in range(n):
                        vmap.append((v, i))
                    cg0 += n
                st[gi] = ((fta, ftb), vmap, True)

        issue_group(0)
        gate_sb = wset.tile([P, BPC * FCH], f32)
        nc.gpsimd.dma_start(gate_sb[:], gt_d[:])
        issue_group(1)
        issue_group(2)

        if nzb:
            ones_bf = const.tile([1, P], bf16)
            nc.vector.memset(ones_bf[:], 1.0)
            s_row = wset.tile([1, TOK], bf16)
            nc.sync.dma_start(s_row[:], s_d[:])
            bc1_sb = wset.tile([1, CH], bf16)
            nc.sync.dma_start(bc1_sb[:], bc1_d[:])
            bc2_sb = wset.tile([1, L], bf16)
            nc.sync.dma_start(bc2_sb[:], bc2_d[:])

        # cls weights (needed from chunk 32 onward; issued after setup DMAs)
        w1_sb = wset.tile([P, (E // P) * CH], bf16)
        nc.sync.dma_start(w1_sb[:], w1_d[:])
        w2_sb = wset.tile([P, (CH // P) * L], bf16)
        nc.sync.dma_start(w2_sb[:], w2_d[:])

        # ---- per-batch cls head ----------------------------------------------
        # Transposed cls dataflow: pooledT [e, q] is accumulated directly in
        # the stream (two halves so half the hT matmuls overlap the stream),
        # hT [ch, q] = Wc1-block^T @ pooledT needs no transposes, relu emits
        # the layout the final matmul wants.
        hT_ps = {}

        def cls_half(b, half, pooledT):
            """Fold one pooled half into hT_ps[b] (32 mms); finish cls on half 1."""
            pTsb = clsp.tile([P, (E // P) * Q], bf16, tag="pTsb",
                             name=f"pTsb{b}_{half}")
            hq = (E // P) * Q // 2
            nc.vector.tensor_copy(pTsb[:, :hq], pooledT[:, :hq])
            nc.vector.tensor_copy(pTsb[:, hq:], pooledT[:, hq:])
            if half == 0:
                hT_ps[b] = aux_ps.tile([P, (CH // P) * Q], f32, tag="aux",
                                       name=f"hT_ps{b}")
                nc.vector.memset(hT_ps[b][:], 0.0)
            hps = hT_ps[b]
            for eb in range(E // P):
                for chb in range(CH // P):
                    nc.tensor.matmul(
                        hps[:, chb * Q:(chb + 1) * Q],
                        w1_sb[:, eb * CH + chb * P:eb * CH + (chb + 1) * P],
                        pTsb[:, eb * Q:(eb + 1) * Q],
                        start=False,
                        stop=(eb == E // P - 1 and half == 1 and not nzb),
                        skip_group_check=True)
            if half == 1 and nzb:
                for chb in range(CH // P):
                    nc.tensor.matmul(hps[:, chb * Q:(chb + 1) * Q],
                                     bc1_sb[:, chb * P:(chb + 1) * P],
                                     ones_bf[:1, :Q], start=False, stop=True,
                                     skip_group_check=True)
            if half == 0:
                return
            hT_sb = clsp.tile([P, (CH // P) * Q], bf16, tag="hT_sb",
                              name=f"hT_sb{b}")
            nc.scalar.activation(hT_sb[:], hps[:], mybir.ActivationFunctionType.Relu)
            o_ps = aux_ps.tile([Q, L], f32, tag="aux", name=f"o_ps{b}")
            for k in range(CH // P):
                nc.tensor.matmul(o_ps[:], hT_sb[:, k * Q:(k + 1) * Q],
                                 w2_sb[:, k * L:(k + 1) * L],
                                 start=(k == 0), stop=(k == CH // P - 1 and not nzb))
            if nzb:
                nc.tensor.matmul(o_ps[:], ones_bf[:1, :Q], bc2_sb[:1, :],
                                 start=False, stop=True)
            o_sb = clsp.tile([Q, L], f32, tag="o_sb", name=f"o_sb{b}")
            nc.vector.tensor_copy(o_sb[:], o_ps[:])
            nc.sync.dma_start(out_d[b * Q:(b + 1) * Q, :], o_sb[:])

        # ---- feature/value stream --------------------------------------------
        # Group-level DMA prefetch (GSKEW groups ahead, first ones issued at
        # setup) + chunk-level compute skew (CSKEW chunks between the logits
        # matmuls and the sigmoid/gate/pooled stage) so the ACT->DVE->PE
        # roundtrip never stalls PE.
        CSKEW = 3
        HFCH = FCH // 2
        nchunks = BPC * FCH
        lgs = {}
        pooled = {}

        for cc in range(nchunks + CSKEW):
            if cc < nchunks:
                if cc % GCH == 0:
                    issue_group(cc // GCH + GSKEW)
                gi, cg = cc // GCH, cc % GCH
                b = cc // FCH
                (fta, ftb), _, split = st[gi]
                ft, fcg = (fta, cg) if (not split or cg < HG) else (ftb, cg - HG)
                lg = lg_ps.tile([P, Q], f32, tag="lg", name=f"lg{cc}")
                for k in range(KF):
                    nc.tensor.matmul(lg[:], ft[:, k, fcg * P:(fcg + 1) * P],
                                     qmwT[:, k * TOK + b * Q:k * TOK + b * Q + Q],
                                     start=(k == 0), stop=(k == KF - 1 and not nzb))
                if nzb:
                    nc.tensor.matmul(lg[:], ones_bf[:1, :P],
                                     s_row[:1, b * Q:(b + 1) * Q],
                                     start=False, stop=True)
                lgs[cc] = lg
            j = cc - CSKEW
            if j >= 0:
                gi, cg = j // GCH, j % GCH
                b, c = j // FCH, j % FCH
                _, vv, split = st[gi]
                if not split:
                    vl, vcg = vv[0], cg
                else:
                    vl, vcg = vv[cg]
                lg = lgs.pop(j)
                sig = sigp.tile([P, Q], f32, tag="sig")
                nc.scalar.activation(sig[:], lg[:],
                                     mybir.ActivationFunctionType.Sigmoid)
                wts = wtsp.tile([P, Q], bf16, tag="wts")
                nc.vector.tensor_scalar_mul(wts[:], sig[:],
                                            gate_sb[:, b * FCH + c:b * FCH + c + 1])
                half = c // HFCH
                if c % HFCH == 0:
                    pooled[(b, half)] = pool_ps.tile([P, (E // P) * Q], f32,
                                                     tag="pooled",
                                                     name=f"pooledT{b}_{half}")
                    nc.vector.memset(pooled[(b, half)][:], 0.0)
                pT = pooled[(b, half)]
                for eb in range(E // P):
                    nc.tensor.matmul(pT[:, eb * Q:(eb + 1) * Q],
                                     vl[:, vcg, eb * P:(eb + 1) * P],
                                     wts[:],
                                     start=False,
                                     stop=(c % HFCH == HFCH - 1),
                                     skip_group_check=True)
                if c % HFCH == HFCH - 1:
                    cls_half(b, half, pT)

    nc.compile()
    return nc


def _fold_weights(inputs):
    """Fold the two affine mappers into C/c0 and the logit-constant u/s0 (float64)."""
    Wq1 = np.asarray(inputs["Wq1"], np.float64)
    Wq2 = np.asarray(inputs["Wq2"], np.float64)
    Wf1 = np.asarray(inputs["Wf1"], np.float64)
    Wf2 = np.asarray(inputs["Wf2"], np.float64)
    bq1 = np.asarray(inputs["bq1"], np.float64)
    bq2 = np.asarray(inputs["bq2"], np.float64)
    bf1 = np.asarray(inputs["bf1"], np.float64)
    bf2 = np.asarray(inputs["bf2"], np.float64)
    T1 = Wq1.T @ Wq2.T                      # (QS, E)
    A = Wf2 @ Wf1                           # (E, FS)
    C = T1 @ A                              # (QS, FS)
    b12 = bq1 @ Wq2.T + bq2                 # (E,)
    c0 = b12 @ A                            # (FS,)
    sv = Wf2 @ bf1 + bf2                    # (E,)
    u = T1 @ sv                             # (QS,)
    s0 = float(b12 @ sv)
    return C, c0, u, s0


def run(inputs, trace=False, tmpdir=None):
    q = np.asarray(inputs["query"], dtype=np.float32)
    feats = np.asarray(inputs["features"], dtype=np.float32)
    vals = np.asarray(inputs["values"], dtype=np.float32)
    ftw = np.asarray(inputs["feature_time_weights"], dtype=np.float32)
    mask = np.asarray(inputs["attention_mask"])
    biases = {k: np.asarray(inputs[k], dtype=np.float32)
              for k in ("bq1", "bq2", "bf1", "bf2", "bc1", "bc2")}
    nzb = any(np.any(v) for v in biases.values())

    if nzb not in _GRAPH_CACHE:
        _GRAPH_CACHE[nzb] = _build(nzb)
    nc = _GRAPH_CACHE[nzb]

    C, c0, u, s0 = _fold_weights(inputs)
    Wc1 = np.asarray(inputs["Wc1"], np.float32)
    Wc2 = np.asarray(inputs["Wc2"], np.float32)

    # effective per-token logit weights (float64 fold, bf16 upload)
    qf = q.reshape(B * Q, QS).astype(np.float64)
    qmw = (qf @ C + c0).astype(np.float32)          # (B*Q, FS)
    s_all = (qf @ u + s0).astype(np.float32)        # (B*Q,)

    # pre-tiled shared weights (see _build dram layouts)
    w1_h = np.ascontiguousarray(
        Wc1.T.astype(bfnp).reshape(E // P, P, CH).transpose(1, 0, 2)
        .reshape(P, (E // P) * CH))
    w2_h = np.ascontiguousarray(
        Wc2.T.astype(bfnp).reshape(CH // P, P, L).transpose(1, 0, 2)
        .reshape(P, (CH // P) * L))
    shared = {"Wc1T": w1_h, "Wc2T": w2_h}
    if nzb:
        shared.update(
            bc1=np.ascontiguousarray(biases["bc1"].astype(bfnp).reshape(1, CH)),
            bc2=np.ascontiguousarray(biases["bc2"].astype(bfnp).reshape(1, L)),
        )

    gate = ftw * mask.astype(np.float32)            # (B, F)
    fbf = feats.astype(bfnp)
    vbf = vals.astype(bfnp)

    in_maps = []
    for cidx in range(N_CORES):
        bs = slice(cidx * BPC, (cidx + 1) * BPC)
        # features: [b, F, FS] -> transposed+tiled [b*g, fs(128), k(4)*1024F]
        fb = fbf[bs].transpose(0, 2, 1)             # (BPC, FS, F)
        ft_h = np.ascontiguousarray(
            fb.reshape(BPC, KF, P, NG, FPG).transpose(0, 3, 2, 1, 4)
            .reshape(BPC * NG, P, KF * FPG))
        # values: [b, F, E] -> [b*g, row-in-chunk(128), cg(8)*E]
        vl_h = np.ascontiguousarray(
            vbf[bs].reshape(BPC, NG, GCH, P, E).transpose(0, 1, 3, 2, 4)
            .reshape(BPC * NG, P, GCH * E))
        # qmwT: (TOK, FS) -> [fs-in-block(128), k(4)*TOK]
        qmwt_h = np.ascontiguousarray(
            qmw[cidx * TOK:(cidx + 1) * TOK].astype(bfnp).T
            .reshape(KF, P, TOK).transpose(1, 0, 2).reshape(P, KF * TOK))
        # gateT: (BPC, F) -> [row-in-chunk(128), b*FCH]
        gt_h = np.ascontiguousarray(
            gate[bs].reshape(BPC, FCH, P).transpose(2, 0, 1).reshape(P, BPC * FCH))
        im = dict(shared, ftT=ft_h, vals=vl_h, qmwT=qmwt_h, gateT=gt_h)
        if nzb:
            im["srow"] = np.ascontiguousarray(
                s_all[cidx * TOK:(cidx + 1) * TOK].astype(bfnp).reshape(1, TOK))
        in_maps.append(im)

    res = run_bass_kernel_spmd(nc, in_maps, core_ids=list(range(N_CORES)),
                               trace=trace, tmpdir=tmpdir)
    out = np.concatenate(
        [res.results[i]["out"].reshape(BPC, Q, L) for i in range(N_CORES)], axis=0)
    return out, res


def kernel(**inputs) -> np.ndarray:
    out, _ = run(inputs, trace=False)
    return out



# revision 3
# speedup vs baseline: 1.2137x; 1.2137x over previous
"""Trainium2 Bass kernel for nn_AttentionEvaluatorModel (8-core SPMD, data-parallel over batch).

Math (reference):
    qm  = (query @ Wq1.T + bq1) @ Wq2.T + bq2                     (B, Q, E)
    fm  = (features @ Wf1.T + bf1) @ Wf2.T + bf2                  (B, F, E)
    wts = sigmoid(qm @ fm.T) * (ftw * mask)                       (B, Q, F)
    out = cls(wts @ values)                                       (B, Q, L)

Restructure: both mappers are affine (no nonlinearity) and fm only appears
inside qm @ fm.T, so the whole query/feature mapper chain folds (float64, on
host) into one effective per-token logit weight:
    qm @ fm.T = qmw @ features.T + s 1^T
    qmw = query @ (Wq1.T Wq2.T Wf2 Wf1) + (bq1 Wq2.T + bq2) Wf2 Wf1   (B*Q, FS)
    s   = query @ (Wq1.T Wq2.T sv) + (bq1 Wq2.T + bq2) sv, sv = Wf2 bf1 + bf2
This removes the (B,F,FS)x(FS,H)x(H,E) feature-mapper matmuls and ~10 MB of
per-core weight DMA. The kernel is then memory-bound on streaming
features+values, which are host-cast to bf16 (halving HBM traffic) and
host-pre-tiled/transposed so every DMA is one instruction with >=1KB
contiguous runs and zero on-chip transposes.

Each of the 8 cores handles B/8 = 2 batches end-to-end; no inter-core comms.

Per-core on-chip dataflow (TOK = 2 batches x 64 queries = 128):
    stream 2x4 groups of [512 fs, 1024 F] features + [1024 F, 512 E] values:
      per 128-F chunk: logits[F,q] = ftT.T @ qmwT   (4 bf16 mm, f32 PSUM)
                       sigmoid (ACT) -> * gate[F]   (DVE, bf16 out)
                       pooledT[e,q] += vl.T @ wts   (4 bf16 mm, f32 PSUM)
    cls in fully transposed orientation (no PE transposes):
      hT[ch,q] = Wc1-blocks.T @ pooledT (accumulated over two pooled halves,
      half overlapping the stream), relu (ACT), out[q,L] = hT.T @ Wc2T.
    The last feature/value group is issued as smaller DMA pieces so the end
    drain after the final bytes is short.

PSUM note: interleaved per-slice accumulation groups clobber each other via
matmul start=True (it resets more than the instruction's own output columns),
so long-lived accumulators (pooledT, hT) are zeroed once with memset and all
their matmuls run start=False + skip_group_check.
"""

import numpy as np
import ml_dtypes
from contextlib import ExitStack

from concourse import bass, bacc, tile, mybir
from concourse.bass_utils import run_bass_kernel_spmd

P = 128
N_CORES = 8
B, Q, F, E = 16, 64, 4096, 512
QS, FS, H, CH, L = 512, 512, 1024, 1024, 128
BPC = B // N_CORES          # batches per core (2)
TOK = BPC * Q               # tokens per core (128)
FCH = F // P                # feature chunks per batch (32)
NG = 4                      # feature groups per batch
FPG = F // NG               # F-rows per group
GCH = FCH // NG             # chunks per group (8)
KQ = QS // P                # 4 contraction blocks over QS
KF = FS // P                # 4 contraction blocks over FS

f32 = mybir.dt.float32
bf16 = mybir.dt.bfloat16
fp8 = mybir.dt.float8e3
bfnp = ml_dtypes.bfloat16
e3np = ml_dtypes.float8_e3m4
VSCALE = 2.0  # values pre-scale into e3m4 normal range; 1/VSCALE folded into Wc1

_GRAPH_CACHE = {}


def _build(nzb: bool):
    """Build the SPMD single-core graph. nzb: whether bias vectors are nonzero."""
    nc = bacc.Bacc("TRN2", target_bir_lowering=False, debug=False,
                   num_devices=N_CORES)

    # host-pre-tiled inputs (see run() for layouts)
    ft_d = nc.dram_tensor("ftT", (BPC * NG, P, KF, FPG), bf16, kind="ExternalInput").ap()
    vl_d = nc.dram_tensor("vals", (BPC * NG, P, GCH, E), fp8, kind="ExternalInput").ap()
    qmwt_d = nc.dram_tensor("qmwT", (P, KF * TOK), bf16, kind="ExternalInput").ap()
    gt_d = nc.dram_tensor("gateT", (P, BPC * FCH), f32, kind="ExternalInput").ap()
    w1_d = nc.dram_tensor("Wc1T", (P, (E // P) * CH), bf16, kind="ExternalInput").ap()
    w2_d = nc.dram_tensor("Wc2T", (P, (CH // P) * L), bf16, kind="ExternalInput").ap()
    if nzb:
        s_d = nc.dram_tensor("srow", (1, TOK), bf16, kind="ExternalInput").ap()
        bc1_d = nc.dram_tensor("bc1", (1, CH), bf16, kind="ExternalInput").ap()
        bc2_d = nc.dram_tensor("bc2", (1, L), bf16, kind="ExternalInput").ap()
    out_d = nc.dram_tensor("out", (TOK, L), f32, kind="ExternalOutput").ap()

    with tile.TileContext(nc) as tc, ExitStack() as ctx:
        const = ctx.enter_context(tc.tile_pool(name="const", bufs=1))
        wset = ctx.enter_context(tc.tile_pool(name="wset", bufs=1))
        ftp = ctx.enter_context(tc.tile_pool(name="ftp", bufs=4))
        valp = ctx.enter_context(tc.tile_pool(name="valp", bufs=4))
        sigp = ctx.enter_context(tc.tile_pool(name="sigp", bufs=4))
        wtsp = ctx.enter_context(tc.tile_pool(name="wtsp", bufs=4))
        clsp = ctx.enter_context(tc.tile_pool(name="clsp", bufs=2))
        aux_ps = ctx.enter_context(tc.tile_pool(name="aux_ps", bufs=1, space="PSUM"))
        lg_ps = ctx.enter_context(tc.tile_pool(name="lg_ps", bufs=4, space="PSUM"))
        pool_ps = ctx.enter_context(tc.tile_pool(name="pool_ps", bufs=3, space="PSUM"))

        # ---- setup DMAs (ordered by first use: qmwT first, then the first
        # feature groups, gate; cls weights much later) ------------------------
        qmwT = wset.tile([P, KF * TOK], bf16)
        nc.gpsimd.dma_start(qmwT[:], qmwt_d[:])
        # stream pools + group DMA issue (the first groups go out right after
        # queryT/C so the feature stream owns the DMA engines from the start)
        GSKEW = 3
        ngroups = BPC * NG
        st = {}

        HG = GCH // 2

        def issue_group(gi):
            if gi >= ngroups:
                return
            if gi < ngroups - 1:
                ft = ftp.tile([P, KF, FPG], bf16, tag="ft")
                nc.sync.dma_start(ft[:], ft_d[gi])
                vl = valp.tile([P, GCH, E], fp8, tag="vl")
                nc.sync.dma_start(vl[:], vl_d[gi])
                st[gi] = ((ft, ft), (vl,), False)
            else:
                # split the last group so the final chunks' tiles (and their
                # DMA semaphores) land earlier, shortening the end drain
                fta = ftp.tile([P, KF, FPG // 2], bf16, tag="ft", name="fta")
                nc.sync.dma_start(fta[:], ft_d[gi, :, :, :FPG // 2])
                pieces = [2, 2, 2, 1, 1]
                vls = []
                vmap = []
                cg0 = 0
                for pi, n in enumerate(pieces):
                    if pi == 1:
                        ftb = ftp.tile([P, KF, FPG // 2], bf16, tag="ft",
                                       name="ftb")
                        nc.sync.dma_start(ftb[:], ft_d[gi, :, :, FPG // 2:])
                    v = valp.tile([P, n, E], fp8, tag="vl", name=f"vl{pi}")
                    nc.sync.dma_start(v[:], vl_d[gi, :, cg0:cg0 + n, :])
                    for i in range(n):
                        vmap.append((v, i))
                    cg0 += n
                st[gi] = ((fta, ftb), vmap, True)

        issue_group(0)
        gate_sb = wset.tile([P, BPC * FCH], f32)
        nc.gpsimd.dma_start(gate_sb[:], gt_d[:])
        issue_group(1)
        issue_group(2)

        if nzb:
            ones_bf = const.tile([1, P], bf16)
            nc.vector.memset(ones_bf[:], 1.0)
            s_row = wset.tile([1, TOK], bf16)
            nc.sync.dma_start(s_row[:], s_d[:])
            bc1_sb = wset.tile([1, CH], bf16)
            nc.sync.dma_start(bc1_sb[:], bc1_d[:])
            bc2_sb = wset.tile([1, L], bf16)
            nc.sync.dma_start(bc2_sb[:], bc2_d[:])

        # cls weights (needed from chunk 32 onward; issued after setup DMAs)
        w1_sb = wset.tile([P, (E // P) * CH], bf16)
        nc.sync.dma_start(w1_sb[:], w1_d[:])
        w2_sb = wset.tile([P, (CH // P) * L], bf16)
        nc.sync.dma_start(w2_sb[:], w2_d[:])

        # ---- per-batch cls head ----------------------------------------------
        # Transposed cls dataflow: pooledT [e, q] is accumulated directly in
        # the stream (two halves so half the hT matmuls overlap the stream),
        # hT [ch, q] = Wc1-block^T @ pooledT needs no transposes, relu emits
        # the layout the final matmul wants.
        hT_ps = {}

        def cls_half(b, half, pooledT):
            """Fold one pooled half into hT_ps[b] (32 mms); finish cls on half 1."""
            pTsb = clsp.tile([P, (E // P) * Q], bf16, tag="pTsb",
                             name=f"pTsb{b}_{half}")
            hq = (E // P) * Q // 2
            nc.vector.tensor_copy(pTsb[:, :hq], pooledT[:, :hq])
            nc.vector.tensor_copy(pTsb[:, hq:], pooledT[:, hq:])
            if half == 0:
                hT_ps[b] = aux_ps.tile([P, (CH // P) * Q], f32, tag="aux",
                                       name=f"hT_ps{b}")
                nc.vector.memset(hT_ps[b][:], 0.0)
            hps = hT_ps[b]
            for eb in range(E // P):
                for chb in range(CH // P):
                    nc.tensor.matmul(
                        hps[:, chb * Q:(chb + 1) * Q],
                        w1_sb[:, eb * CH + chb * P:eb * CH + (chb + 1) * P],
                        pTsb[:, eb * Q:(eb + 1) * Q],
                        start=False,
                        stop=(eb == E // P - 1 and half == 1 and not nzb),
                        skip_group_check=True)
            if half == 1 and nzb:
                for chb in range(CH // P):
                    nc.tensor.matmul(hps[:, chb * Q:(chb + 1) * Q],
                                     bc1_sb[:, chb * P:(chb + 1) * P],
                                     ones_bf[:1, :Q], start=False, stop=True,
                                     skip_group_check=True)
            if half == 0:
                return
            hT_sb = clsp.tile([P, (CH // P) * Q], bf16, tag="hT_sb",
                              name=f"hT_sb{b}")
            nc.scalar.activation(hT_sb[:], hps[:], mybir.ActivationFunctionType.Relu)
            o_ps = aux_ps.tile([Q, L], f32, tag="aux", name=f"o_ps{b}")
            for k in range(CH // P):
                nc.tensor.matmul(o_ps[:], hT_sb[:, k * Q:(k + 1) * Q],
                                 w2_sb[:, k * L:(k + 1) * L],
                                 start=(k == 0), stop=(k == CH // P - 1 and not nzb))
            if nzb:
                nc.tensor.matmul(o_ps[:], ones_bf[:1, :Q], bc2_sb[:1, :],
                                 start=False, stop=True)
            o_sb = clsp.tile([Q, L], f32, tag="o_sb", name=f"o_sb{b}")
            nc.vector.tensor_copy(o_sb[:], o_ps[:])
            nc.sync.dma_start(out_d[b * Q:(b + 1) * Q, :], o_sb[:])

        # ---- feature/value stream --------------------------------------------
        # Group-level DMA prefetch (GSKEW groups ahead, first ones issued at
        # setup) + chunk-level compute skew (CSKEW chunks between the logits
        # matmuls and the sigmoid/gate/pooled stage) so the ACT->DVE->PE
        # roundtrip never stalls PE.
        CSKEW = 3
        HFCH = FCH // 2
        nchunks = BPC * FCH
        lgs = {}
        pooled = {}

        for cc in range(nchunks + CSKEW):
            if cc < nchunks:
                if cc % GCH == 0:
                    issue_group(cc // GCH + GSKEW)
                gi, cg = cc // GCH, cc % GCH
                b = cc // FCH
                (fta, ftb), _, split = st[gi]
                ft, fcg = (fta, cg) if (not split or cg < HG) else (ftb, cg - HG)
                lg = lg_ps.tile([P, Q], f32, tag="lg", name=f"lg{cc}")
                for k in range(KF):
                    nc.tensor.matmul(lg[:], ft[:, k, fcg * P:(fcg + 1) * P],
                                     qmwT[:, k * TOK + b * Q:k * TOK + b * Q + Q],
                                     start=(k == 0), stop=(k == KF - 1 and not nzb))
                if nzb:
                    nc.tensor.matmul(lg[:], ones_bf[:1, :P],
                                     s_row[:1, b * Q:(b + 1) * Q],
                                     start=False, stop=True)
                lgs[cc] = lg
            j = cc - CSKEW
            if j >= 0:
                gi, cg = j // GCH, j % GCH
                b, c = j // FCH, j % FCH
                _, vv, split = st[gi]
                if not split:
                    vl, vcg = vv[0], cg
                else:
                    vl, vcg = vv[cg]
                lg = lgs.pop(j)
                sig = sigp.tile([P, Q], f32, tag="sig")
                nc.scalar.activation(sig[:], lg[:],
                                     mybir.ActivationFunctionType.Sigmoid)
                wts = wtsp.tile([P, Q], bf16, tag="wts")
                nc.vector.tensor_scalar_mul(wts[:], sig[:],
                                            gate_sb[:, b * FCH + c:b * FCH + c + 1])
                half = c // HFCH
                if c % HFCH == 0:
                    pooled[(b, half)] = pool_ps.tile([P, (E // P) * Q], f32,
                                                     tag="pooled",
                                                     name=f"pooledT{b}_{half}")
                    nc.vector.memset(pooled[(b, half)][:], 0.0)
                pT = pooled[(b, half)]
                for eb in range(E // P):
                    nc.tensor.matmul(pT[:, eb * Q:(eb + 1) * Q],
                                     vl[:, vcg, eb * P:(eb + 1) * P],
                                     wts[:],
                                     start=False,
                                     stop=(c % HFCH == HFCH - 1),
                                     skip_group_check=True)
                if c % HFCH == HFCH - 1:
                    cls_half(b, half, pT)

    nc.compile()
    return nc


def _fold_weights(inputs):
    """Fold the two affine mappers into C/c0 and the logit-constant u/s0 (float64)."""
    Wq1 = np.asarray(inputs["Wq1"], np.float64)
    Wq2 = np.asarray(inputs["Wq2"], np.float64)
    Wf1 = np.asarray(inputs["Wf1"], np.float64)
    Wf2 = np.asarray(inputs["Wf2"], np.float64)
    bq1 = np.asarray(inputs["bq1"], np.float64)
    bq2 = np.asarray(inputs["bq2"], np.float64)
    bf1 = np.asarray(inputs["bf1"], np.float64)
    bf2 = np.asarray(inputs["bf2"], np.float64)
    T1 = Wq1.T @ Wq2.T                      # (QS, E)
    A = Wf2 @ Wf1                           # (E, FS)
    C = T1 @ A                              # (QS, FS)
    b12 = bq1 @ Wq2.T + bq2                 # (E,)
    c0 = b12 @ A                            # (FS,)
    sv = Wf2 @ bf1 + bf2                    # (E,)
    u = T1 @ sv                             # (QS,)
    s0 = float(b12 @ sv)
    return C, c0, u, s0


def run(inputs, trace=False, tmpdir=None):
    q = np.asarray(inputs["query"], dtype=np.float32)
    feats = np.asarray(inputs["features"], dtype=np.float32)
    vals = np.asarray(inputs["values"], dtype=np.float32)
    ftw = np.asarray(inputs["feature_time_weights"], dtype=np.float32)
    mask = np.asarray(inputs["attention_mask"])
    biases = {k: np.asarray(inputs[k], dtype=np.float32)
              for k in ("bq1", "bq2", "bf1", "bf2", "bc1", "bc2")}
    nzb = any(np.any(v) for v in biases.values())

    if nzb not in _GRAPH_CACHE:
        _GRAPH_CACHE[nzb] = _build(nzb)
    nc = _GRAPH_CACHE[nzb]

    C, c0, u, s0 = _fold_weights(inputs)
    Wc1 = np.asarray(inputs["Wc1"], np.float32)
    Wc2 = np.asarray(inputs["Wc2"], np.float32)

    # effective per-token logit weights (float64 fold, bf16 upload)
    qf = q.reshape(B * Q, QS).astype(np.float64)
    qmw = (qf @ C + c0).astype(np.float32)          # (B*Q, FS)
    s_all = (qf @ u + s0).astype(np.float32)        # (B*Q,)

    # pre-tiled shared weights (see _build dram layouts)
    w1_h = np.ascontiguousarray(
        (Wc1.T / VSCALE).astype(bfnp).reshape(E // P, P, CH).transpose(1, 0, 2)
        .reshape(P, (E // P) * CH))
    w2_h = np.ascontiguousarray(
        Wc2.T.astype(bfnp).reshape(CH // P, P, L).transpose(1, 0, 2)
        .reshape(P, (CH // P) * L))
    shared = {"Wc1T": w1_h, "Wc2T": w2_h}
    if nzb:
        shared.update(
            bc1=np.ascontiguousarray(biases["bc1"].astype(bfnp).reshape(1, CH)),
            bc2=np.ascontiguousarray(biases["bc2"].astype(bfnp).reshape(1, L)),
        )

    gate = ftw * mask.astype(np.float32)            # (B, F)
    fbf = feats.astype(bfnp)
    vbf = (vals * np.float32(VSCALE)).astype(e3np)

    in_maps = []
    for cidx in range(N_CORES):
        bs = slice(cidx * BPC, (cidx + 1) * BPC)
        # features: [b, F, FS] -> transposed+tiled [b*g, fs(128), k(4)*1024F]
        fb = fbf[bs].transpose(0, 2, 1)             # (BPC, FS, F)
        ft_h = np.ascontiguousarray(
            fb.reshape(BPC, KF, P, NG, FPG).transpose(0, 3, 2, 1, 4)
            .reshape(BPC * NG, P, KF * FPG))
        # values: [b, F, E] -> [b*g, row-in-chunk(128), cg(8)*E]
        vl_h = np.ascontiguousarray(
            vbf[bs].reshape(BPC, NG, GCH, P, E).transpose(0, 1, 3, 2, 4)
            .reshape(BPC * NG, P, GCH * E))
        # qmwT: (TOK, FS) -> [fs-in-block(128), k(4)*TOK]
        qmwt_h = np.ascontiguousarray(
            qmw[cidx * TOK:(cidx + 1) * TOK].astype(bfnp).T
            .reshape(KF, P, TOK).transpose(1, 0, 2).reshape(P, KF * TOK))
        # gateT: (BPC, F) -> [row-in-chunk(128), b*FCH]
        gt_h = np.ascontiguousarray(
            gate[bs].reshape(BPC, FCH, P).transpose(2, 0, 1).reshape(P, BPC * FCH))
        im = dict(shared, ftT=ft_h, vals=vl_h, qmwT=qmwt_h, gateT=gt_h)
        if nzb:
            im["srow"] = np.ascontiguousarray(
                s_all[cidx * TOK:(cidx + 1) * TOK].astype(bfnp).reshape(1, TOK))
        in_maps.append(im)

    res = run_bass_kernel_spmd(nc, in_maps, core_ids=list(range(N_CORES)),
                               trace=trace, tmpdir=tmpdir)
    out = np.concatenate(
        [res.results[i]["out"].reshape(BPC, Q, L) for i in range(N_CORES)], axis=0)
    return out, res


def kernel(**inputs) -> np.ndarray:
    out, _ = run(inputs, trace=False)
    return out

